# revision 3
# baseline (speedup 1.0000x reference)
"""Trainium2 Bass kernel for the ExpandFrame (TTS duration-expansion) module.

Math (per batch b):
    c[l]   = cumsum(duration)[l] - 0.5*round(duration[l])        # segment centers
    w[l,t] = exp(temp * (t - c[l])^2),  temp = -1/(5*sqrt(duration[0,0]))
    out[d,t] = sum_l w[l,t]*hidden[l,d] / sum_l w[l,t]

Key structure exploited: temp ~ -0.09, so w underflows to exactly 0 in fp32
for |t - c| > ~31. c is monotonically increasing (~4 per phoneme), so each
128-frame chunk of t only interacts with a ~100-phoneme band of l. One
K<=128 matmul per chunk covers the whole contraction; a second N=1 matmul
with a ones vector (same stationary weights) produces the denominator.
Normalization happens after the matmul (it's a per-frame scalar).

Sharding: data-parallel over batch B=8, one batch element per NeuronCore.

Tail handling: frames beyond a batch row's total duration have all-zero
weights in fp32 -> reference emits NaN (0/0). Frames slightly before that
have denormal-range denominators. We apply a per-frame exponent shift
(log-sum-exp style) on the last couple of chunks so the device math stays
in normal fp32 range, and multiply in a host-computed {1.0, NaN} mask to
reproduce the reference's NaN pattern exactly.
"""

import numpy as np

# ---------------------------------------------------------------- constants
B, L, D = 8, 1024, 512
TCHUNK = 128          # frames per output chunk (psum partition dim)
GROUP = 2             # chunks sharing one hidden-window DMA
R_MARGIN = 40.0       # band margin in t units (w==0 beyond |d|>31.1)
DANGER_S = -60.0      # frames with max-x below this use the shift path
LN_CUTOFF = float(np.log(2.0 ** -126))  # XLA exp flushes results below 2^-126
N_CORES = 8
MM_F32R = False       # use float32r (full-rate) matmuls instead of fp32


# ---------------------------------------------------------------- host prep
def _host_prep(duration):
    """c [B,L] f32 and temp f32, bit-matching the jax-cpu reference."""
    try:
        import jax
        import jax.numpy as jnp

        def c_fn(d):
            e = jnp.cumsum(d, axis=-1)
            return e - 0.5 * jnp.round(d), -1.0 / (5.0 * jnp.sqrt(d[0, 0]))

        c, temp = jax.jit(c_fn, backend="cpu")(duration)
        return np.asarray(c), np.float32(np.asarray(temp))
    except Exception:
        e = np.cumsum(duration.astype(np.float64), axis=-1)
        c = (e - 0.5 * np.round(duration.astype(np.float64))).astype(np.float32)
        temp = np.float32(-1.0 / (5.0 * np.sqrt(np.float64(duration[0, 0]))))
        return c, temp


def _plan(c, temp, T):
    """Chunk/group/window plan (shared across cores: unions over B)."""
    nT = (T + TCHUNK - 1) // TCHUNK
    chunks = [(n * TCHUNK, min(TCHUNK, T - n * TCHUNK)) for n in range(nT)]

    # per-frame max_l x = temp * (distance to nearest c)^2, per core
    t = np.arange(T, dtype=np.float64)
    s_rows = np.zeros((B, T), np.float32)
    for b in range(B):
        cb = c[b].astype(np.float64)
        idx = np.searchsorted(cb, t)
        lo = np.clip(idx - 1, 0, L - 1)
        hi = np.clip(idx, 0, L - 1)
        d = np.minimum(np.abs(t - cb[lo]), np.abs(t - cb[hi]))
        s_rows[b] = (float(temp) * d * d).astype(np.float32)

    s_min = s_rows.min(axis=0)
    danger = {
        n for n, (t0, m) in enumerate(chunks) if (s_min[t0 : t0 + m] < DANGER_S).any()
    }

    # NaN where the reference denominator is exactly zero in fp32
    nan_mask = np.where(s_rows < LN_CUTOFF, np.float32(np.nan), np.float32(1.0))

    # shift rows, applied only on danger chunks; stored pre-divided by temp
    s_div = np.zeros_like(s_rows)
    for n in danger:
        t0, m = chunks[n]
        s_div[:, t0 : t0 + m] = s_rows[:, t0 : t0 + m] / np.float32(temp)

    def _band(t_lo, t_hi):
        l_lo, l_hi = L, -1
        for b in range(B):
            l_lo = min(l_lo, int(np.searchsorted(c[b], np.float32(t_lo))))
            l_hi = max(l_hi, int(np.searchsorted(c[b], np.float32(t_hi))))
        l_lo = max(0, min(l_lo, L - 1))
        l_hi = min(max(l_hi, l_lo + 1), L)
        return l_lo, l_hi

    groups = []
    for g0 in range(0, nT, GROUP):
        g_chunks = list(range(g0, min(g0 + GROUP, nT)))
        t_lo = chunks[g_chunks[0]][0] - R_MARGIN
        t_hi = chunks[g_chunks[-1]][0] + chunks[g_chunks[-1]][1] + R_MARGIN
        l_lo, l_hi = _band(t_lo, t_hi)
        windows = []
        for wl0 in range(l_lo, l_hi, 128):
            k = min(128, l_hi - wl0)
            windows.append((min(wl0, L - k), k))
        chunk_windows = {}
        for ci in g_chunks:
            t0, m = chunks[ci]
            cl_lo, cl_hi = _band(t0 - R_MARGIN, t0 + m + R_MARGIN)
            wins = [
                wi for wi, (wl0, k) in enumerate(windows)
                if wl0 < cl_hi and wl0 + k > cl_lo
            ]
            chunk_windows[ci] = wins or [len(windows) - 1]
        groups.append((windows, g_chunks, chunk_windows))
    return chunks, groups, danger, s_div, nan_mask


# ---------------------------------------------------------------- device IR
def _build_program(T, temp, chunks, groups, danger):
    import concourse.bass as bass
    import concourse.tile as tile
    from concourse import bacc, mybir

    nc = bacc.Bacc("TRN2", target_bir_lowering=False, debug=False)
    f32 = mybir.dt.float32
    mm_dt = mybir.dt.float32r if MM_F32R else f32

    hidden_d = nc.dram_tensor("hidden", [L, D], f32, kind="ExternalInput").ap()
    cneg_d = nc.dram_tensor("cneg", [L], f32, kind="ExternalInput").ap()
    srow_d = nc.dram_tensor("srow", [T], f32, kind="ExternalInput").ap()
    nmask_d = nc.dram_tensor("nmask", [T], f32, kind="ExternalInput").ap()
    iota_d = nc.dram_tensor("iota", [TCHUNK], f32, kind="ExternalInput").ap()
    out_d = nc.dram_tensor("out", [T, D], f32, kind="ExternalOutput").ap()

    def bcast(ap_1d, parts):
        """[n] AP -> [parts, n] AP with partition stride 0 (DMA broadcast)."""
        return bass.AP(tensor=ap_1d.tensor, offset=ap_1d.offset,
                       ap=[[0, parts]] + list(ap_1d.ap))

    Square = mybir.ActivationFunctionType.Square
    Exp = mybir.ActivationFunctionType.Exp

    with tile.TileContext(nc) as tc:
        with (
            tc.tile_pool(name="singles", bufs=1) as singles,
            tc.tile_pool(name="hp", bufs=3) as hp,
            tc.tile_pool(name="cp", bufs=3) as cp,
            tc.tile_pool(name="bp", bufs=4) as bp,
            tc.tile_pool(name="sqp", bufs=3) as sqp,
            tc.tile_pool(name="wp", bufs=3) as wp,
            tc.tile_pool(name="sp", bufs=2) as sp,
            tc.tile_pool(name="rp", bufs=4) as rp,
            tc.tile_pool(name="mp", bufs=2) as mp,
            tc.tile_pool(name="outp", bufs=3) as outp,
            tc.tile_pool(name="pso", bufs=3, space="PSUM") as pso,
            tc.tile_pool(name="psd", bufs=3, space="PSUM") as psd,
        ):
            iota_t = singles.tile([TCHUNK, TCHUNK], f32)
            nc.sync.dma_start(out=iota_t[:], in_=bcast(iota_d, TCHUNK))
            ones_t = singles.tile([TCHUNK, 1], mm_dt)
            nc.vector.memset(ones_t[:], 1.0)

            for windows, g_chunks, chunk_windows in groups:
                h_ts, cn_ts = [], []
                for (l0, k) in windows:
                    h_t = hp.tile([TCHUNK, D], mm_dt)
                    nc.sync.dma_start(out=h_t[:k, :], in_=hidden_d[l0 : l0 + k, :])
                    cn_t = cp.tile([TCHUNK, 1], f32)
                    nc.sync.dma_start(
                        out=cn_t[:k, :], in_=cneg_d[l0 : l0 + k, None]
                    )
                    h_ts.append(h_t)
                    cn_ts.append(cn_t)

                for ci in g_chunks:
                    t0, m = chunks[ci]
                    is_danger = ci in danger
                    po = pso.tile([TCHUNK, D], f32)
                    pd = psd.tile([TCHUNK, 1], f32)
                    if is_danger:
                        s_t = sp.tile([TCHUNK, TCHUNK], f32)
                        nc.sync.dma_start(
                            out=s_t[:, :m], in_=bcast(srow_d[t0 : t0 + m], TCHUNK)
                        )
                    wins = chunk_windows[ci]
                    for j, wi in enumerate(wins):
                        l0, k = windows[wi]
                        bias = bp.tile([TCHUNK, 1], f32)
                        nc.vector.tensor_scalar_add(
                            bias[:k, :], cn_ts[wi][:k, :], float(t0)
                        )
                        sq = sqp.tile([TCHUNK, TCHUNK], f32)
                        nc.scalar.activation(
                            out=sq[:k, :m], in_=iota_t[:k, :m], func=Square,
                            bias=bias[:k, :], scale=1.0,
                        )
                        if is_danger:
                            us = sqp.tile([TCHUNK, TCHUNK], f32)
                            nc.vector.tensor_sub(
                                us[:k, :m], sq[:k, :m], s_t[:k, :m]
                            )
                            sq = us
                        w_t = wp.tile([TCHUNK, TCHUNK], mm_dt)
                        nc.scalar.activation(
                            out=w_t[:k, :m], in_=sq[:k, :m], func=Exp,
                            scale=float(temp),
                        )
                        nc.tensor.matmul(
                            po[:m, :], lhsT=w_t[:k, :m], rhs=h_ts[wi][:k, :],
                            start=(j == 0), stop=(j == len(wins) - 1),
                        )
                        nc.tensor.matmul(
                            pd[:m, :], lhsT=w_t[:k, :m], rhs=ones_t[:k, :],
                            start=(j == 0), stop=(j == len(wins) - 1),
                        )
                    recip = rp.tile([TCHUNK, 1], f32)
                    nc.vector.reciprocal(out=recip[:m, :], in_=pd[:m, :])
                    if is_danger:
                        msk = mp.tile([TCHUNK, 1], f32)
                        nc.sync.dma_start(
                            out=msk[:m, :], in_=nmask_d[t0 : t0 + m, None]
                        )
                        r2 = rp.tile([TCHUNK, 1], f32)
                        nc.vector.tensor_mul(r2[:m, :], recip[:m, :], msk[:m, :])
                        recip = r2
                    ot = outp.tile([TCHUNK, D], f32)
                    nc.vector.tensor_scalar_mul(ot[:m, :], po[:m, :], recip[:m, :])
                    nc.sync.dma_start(out=out_d[t0 : t0 + m, :], in_=ot[:m, :])
    nc.compile()
    return nc


# ---------------------------------------------------------------- entry
def kernel(hidden, duration, t_max):
    from concourse.bass_utils import run_bass_kernel_spmd

    hidden = np.ascontiguousarray(np.asarray(hidden, dtype=np.float32))
    duration = np.ascontiguousarray(np.asarray(duration, dtype=np.float32))
    T = int(t_max)

    c, temp = _host_prep(duration)
    chunks, groups, danger, s_div, nan_mask = _plan(c, temp, T)
    nc = _build_program(T, temp, chunks, groups, danger)

    iota = np.arange(TCHUNK, dtype=np.float32)
    in_maps = [
        {
            "hidden": hidden[b],
            "cneg": -c[b],
            "srow": s_div[b],
            "nmask": nan_mask[b],
            "iota": iota,
        }
        for b in range(B)
    ]
    res = run_bass_kernel_spmd(nc, in_maps, core_ids=list(range(N_CORES))).results
    out = np.empty((B, D, T), np.float32)
    for b in range(B):
        out[b] = res[b]["out"].T
    return out


# revision 19
# speedup vs baseline: 1.1621x; 1.1621x over previous
"""Trainium2 Bass kernel for the ExpandFrame (TTS duration-expansion) module.

Math (per batch b):
    c[l]   = cumsum(duration)[l] - 0.5*round(duration[l])        # segment centers
    w[l,t] = exp(temp * (t - c[l])^2),  temp = -1/(5*sqrt(duration[0,0]))
    out[d,t] = sum_l w[l,t]*hidden[l,d] / sum_l w[l,t]

Key structure exploited: temp ~ -0.09, so w underflows to exactly 0 in fp32
for |t - c| > ~31. c is monotonically increasing (~4 per phoneme), so each
128-frame chunk of t only interacts with a ~100-phoneme band of l. One
K<=128 matmul per chunk covers the whole contraction; a second N=1 matmul
with a ones vector (same stationary weights) produces the denominator.
Normalization happens after the matmul (it's a per-frame scalar).

Sharding: data-parallel over batch B=8, one batch element per NeuronCore.

Tail handling: frames beyond a batch row's total duration have all-zero
weights in fp32 -> reference emits NaN (0/0). Frames slightly before that
have denormal-range denominators. We apply a per-frame exponent shift
(log-sum-exp style) on the last couple of chunks so the device math stays
in normal fp32 range, and multiply in a host-computed {1.0, NaN} mask to
reproduce the reference's NaN pattern exactly.
"""

import numpy as np

# ---------------------------------------------------------------- constants
B, L, D = 8, 1024, 512
TCHUNK = 128          # frames per output chunk (psum partition dim)
GROUP = 2             # chunks sharing one hidden-window DMA
R_MARGIN = 40.0       # band margin in t units (w==0 beyond |d|>31.1)
DANGER_S = -60.0      # frames with max-x below this use the shift path
LN_CUTOFF = float(np.log(2.0 ** -126))  # XLA exp flushes results below 2^-126
N_CORES = 8
MM_F32R = True        # use float32r (full-rate) matmuls instead of fp32


# ---------------------------------------------------------------- host prep
def _host_prep(duration):
    """c [B,L] f32 and temp f32, bit-matching the jax-cpu reference."""
    try:
        import jax
        import jax.numpy as jnp

        def c_fn(d):
            e = jnp.cumsum(d, axis=-1)
            return e - 0.5 * jnp.round(d), -1.0 / (5.0 * jnp.sqrt(d[0, 0]))

        c, temp = jax.jit(c_fn, backend="cpu")(duration)
        return np.asarray(c), np.float32(np.asarray(temp))
    except Exception:
        e = np.cumsum(duration.astype(np.float64), axis=-1)
        c = (e - 0.5 * np.round(duration.astype(np.float64))).astype(np.float32)
        temp = np.float32(-1.0 / (5.0 * np.sqrt(np.float64(duration[0, 0]))))
        return c, temp


def _plan(c, temp, T):
    """Chunk/group/window plan (shared across cores: unions over B)."""
    nT = (T + TCHUNK - 1) // TCHUNK
    chunks = [(n * TCHUNK, min(TCHUNK, T - n * TCHUNK)) for n in range(nT)]

    # per-frame max_l x = temp * (distance to nearest c)^2, per core
    t = np.arange(T, dtype=np.float64)
    s_rows = np.zeros((B, T), np.float32)
    for b in range(B):
        cb = c[b].astype(np.float64)
        idx = np.searchsorted(cb, t)
        lo = np.clip(idx - 1, 0, L - 1)
        hi = np.clip(idx, 0, L - 1)
        d = np.minimum(np.abs(t - cb[lo]), np.abs(t - cb[hi]))
        s_rows[b] = (float(temp) * d * d).astype(np.float32)

    s_min = s_rows.min(axis=0)
    danger = {
        n for n, (t0, m) in enumerate(chunks) if (s_min[t0 : t0 + m] < DANGER_S).any()
    }

    # NaN where the reference denominator is exactly zero in fp32
    nan_mask = np.where(s_rows < LN_CUTOFF, np.float32(np.nan), np.float32(1.0))

    # shift rows, applied only on danger chunks; stored pre-divided by temp
    s_div = np.zeros_like(s_rows)
    for n in danger:
        t0, m = chunks[n]
        s_div[:, t0 : t0 + m] = s_rows[:, t0 : t0 + m] / np.float32(temp)

    def _band(t_lo, t_hi):
        l_lo, l_hi = L, -1
        for b in range(B):
            l_lo = min(l_lo, int(np.searchsorted(c[b], np.float32(t_lo))))
            l_hi = max(l_hi, int(np.searchsorted(c[b], np.float32(t_hi))))
        l_lo = max(0, min(l_lo, L - 1))
        l_hi = min(max(l_hi, l_lo + 1), L)
        return l_lo, l_hi

    groups = []
    for g0 in range(0, nT, GROUP):
        g_chunks = list(range(g0, min(g0 + GROUP, nT)))
        t_lo = chunks[g_chunks[0]][0] - R_MARGIN
        t_hi = chunks[g_chunks[-1]][0] + chunks[g_chunks[-1]][1] + R_MARGIN
        l_lo, l_hi = _band(t_lo, t_hi)
        windows = []
        for wl0 in range(l_lo, l_hi, 128):
            k = min(128, l_hi - wl0)
            windows.append((min(wl0, L - k), k))
        chunk_windows = {}
        for ci in g_chunks:
            t0, m = chunks[ci]
            cl_lo, cl_hi = _band(t0 - R_MARGIN, t0 + m + R_MARGIN)
            wins = [
                wi for wi, (wl0, k) in enumerate(windows)
                if wl0 < cl_hi and wl0 + k > cl_lo
            ]
            chunk_windows[ci] = wins or [len(windows) - 1]
        groups.append((windows, g_chunks, chunk_windows))
    return chunks, groups, danger, s_div, nan_mask


# ---------------------------------------------------------------- device IR
def _build_program(T, temp, chunks, groups, danger):
    import concourse.bass as bass
    import concourse.tile as tile
    from concourse import bacc, mybir

    nc = bacc.Bacc("TRN2", target_bir_lowering=False, debug=False)
    f32 = mybir.dt.float32
    mm_dt = mybir.dt.float32r if MM_F32R else f32

    hidden_d = nc.dram_tensor("hidden", [L, D], mm_dt, kind="ExternalInput").ap()
    cneg_d = nc.dram_tensor("cneg", [L], f32, kind="ExternalInput").ap()
    srow_d = nc.dram_tensor("srow", [T], f32, kind="ExternalInput").ap()
    nmask_d = nc.dram_tensor("nmask", [T], f32, kind="ExternalInput").ap()
    iota_d = nc.dram_tensor("iota", [TCHUNK], f32, kind="ExternalInput").ap()
    ones_d = nc.dram_tensor("ones", [TCHUNK, 2], mm_dt, kind="ExternalInput").ap()
    out_d = nc.dram_tensor("out", [T, D], f32, kind="ExternalOutput").ap()

    def bcast(ap_1d, parts):
        """[n] AP -> [parts, n] AP with partition stride 0 (DMA broadcast)."""
        return bass.AP(tensor=ap_1d.tensor, offset=ap_1d.offset,
                       ap=[[0, parts]] + list(ap_1d.ap))

    Square = mybir.ActivationFunctionType.Square
    Exp = mybir.ActivationFunctionType.Exp

    with tile.TileContext(nc) as tc:
        with (
            tc.tile_pool(name="singles", bufs=1) as singles,
            tc.tile_pool(name="hp", bufs=3) as hp,
            tc.tile_pool(name="cp", bufs=3) as cp,
            tc.tile_pool(name="bp", bufs=4) as bp,
            tc.tile_pool(name="sqp", bufs=3) as sqp,
            tc.tile_pool(name="wp", bufs=3) as wp,
            tc.tile_pool(name="sp", bufs=2) as sp,
            tc.tile_pool(name="rp", bufs=4) as rp,
            tc.tile_pool(name="mp", bufs=2) as mp,
            tc.tile_pool(name="outp", bufs=3) as outp,
            tc.tile_pool(name="pso", bufs=3, space="PSUM") as pso,
            tc.tile_pool(name="psd", bufs=3, space="PSUM") as psd,
        ):
            iota_t = singles.tile([TCHUNK, TCHUNK], f32)
            nc.sync.dma_start(out=iota_t[:], in_=bcast(iota_d, TCHUNK))
            ones_t = singles.tile([TCHUNK, 2], mm_dt)
            nc.sync.dma_start(out=ones_t[:], in_=ones_d[:, :])

            for windows, g_chunks, chunk_windows in groups:
                h_ts, cn_ts = [], []
                for (l0, k) in windows:
                    h_t = hp.tile([TCHUNK, D], mm_dt)
                    nc.sync.dma_start(out=h_t[:k, :], in_=hidden_d[l0 : l0 + k, :])
                    cn_t = cp.tile([TCHUNK, 1], f32)
                    nc.sync.dma_start(
                        out=cn_t[:k, :], in_=cneg_d[l0 : l0 + k, None]
                    )
                    h_ts.append(h_t)
                    cn_ts.append(cn_t)

                for ci in g_chunks:
                    t0, m = chunks[ci]
                    is_danger = ci in danger
                    po = pso.tile([TCHUNK, D], f32)
                    pd = psd.tile([TCHUNK, 2], f32)
                    if is_danger:
                        s_t = sp.tile([TCHUNK, TCHUNK], f32)
                        nc.sync.dma_start(
                            out=s_t[:, :m], in_=bcast(srow_d[t0 : t0 + m], TCHUNK)
                        )
                    wins = chunk_windows[ci]
                    for j, wi in enumerate(wins):
                        l0, k = windows[wi]
                        bias = bp.tile([TCHUNK, 1], f32)
                        nc.vector.tensor_scalar_add(
                            bias[:k, :], cn_ts[wi][:k, :], float(t0)
                        )
                        sq = sqp.tile([TCHUNK, TCHUNK], f32)
                        nc.scalar.activation(
                            out=sq[:k, :m], in_=iota_t[:k, :m], func=Square,
                            bias=bias[:k, :], scale=1.0,
                        )
                        if is_danger:
                            us = sqp.tile([TCHUNK, TCHUNK], f32)
                            nc.vector.tensor_sub(
                                us[:k, :m], sq[:k, :m], s_t[:k, :m]
                            )
                            sq = us
                        w_t = wp.tile([TCHUNK, TCHUNK], mm_dt)
                        nc.scalar.activation(
                            out=w_t[:k, :m], in_=sq[:k, :m], func=Exp,
                            scale=float(temp),
                        )
                        nc.tensor.matmul(
                            po[:m, :], lhsT=w_t[:k, :m], rhs=h_ts[wi][:k, :],
                            start=(j == 0), stop=(j == len(wins) - 1),
                        )
                        nc.tensor.matmul(
                            pd[:m, :], lhsT=w_t[:k, :m], rhs=ones_t[:k, :],
                            start=(j == 0), stop=(j == len(wins) - 1),
                        )
                    recip = rp.tile([TCHUNK, 1], f32)
                    nc.vector.reciprocal(out=recip[:m, :], in_=pd[:m, 0:1])
                    if is_danger:
                        msk = mp.tile([TCHUNK, 1], f32)
                        nc.sync.dma_start(
                            out=msk[:m, :], in_=nmask_d[t0 : t0 + m, None]
                        )
                        r2 = rp.tile([TCHUNK, 1], f32)
                        nc.vector.tensor_mul(r2[:m, :], recip[:m, :], msk[:m, :])
                        recip = r2
                    ot = outp.tile([TCHUNK, D], f32)
                    nc.vector.tensor_scalar_mul(ot[:m, :], po[:m, :], recip[:m, :])
                    nc.sync.dma_start(out=out_d[t0 : t0 + m, :], in_=ot[:m, :])
    nc.compile()
    return nc


# ---------------------------------------------------------------- entry
def kernel(hidden, duration, t_max):
    from concourse.bass_utils import run_bass_kernel_spmd

    hidden = np.ascontiguousarray(np.asarray(hidden, dtype=np.float32))
    duration = np.ascontiguousarray(np.asarray(duration, dtype=np.float32))
    T = int(t_max)

    c, temp = _host_prep(duration)
    chunks, groups, danger, s_div, nan_mask = _plan(c, temp, T)
    nc = _build_program(T, temp, chunks, groups, danger)

    iota = np.arange(TCHUNK, dtype=np.float32)
    in_maps = [
        {
            "hidden": hidden[b],
            "cneg": -c[b],
            "srow": s_div[b],
            "nmask": nan_mask[b],
            "iota": iota,
            "ones": np.ones((TCHUNK, 2), np.float32),
        }
        for b in range(B)
    ]
    res = run_bass_kernel_spmd(nc, in_maps, core_ids=list(range(N_CORES))).results
    out = np.empty((B, D, T), np.float32)
    for b in range(B):
        out[b] = res[b]["out"].T
    return out
